# revision 10
# baseline (speedup 1.0000x reference)
"""Trainium2 Bass kernel for the pixel-RNN (tanh RNN, T=784, H=512, B=256).

Strategy: data-parallel over batch (32 samples per core, 8 cores).

Per core the recurrence out[i,b] = sum_j W[i,j] h[b,j] + w_ih[i] x[b,t] is
computed W-STATIONARY with full-array [128,128] weight tiles (fp16 -> Fast
Weight Load; a [128,128] LDW + N=32 matmul slot measures ~42ns, same as a
32-col slot, so fewer/bigger slots win).  Output lands directly in hT layout
[i on partitions, b free], so NO transposes: tanh on ScalarE (PSUM -> SBUF,
fp16) feeds the next step's moving operand directly.

Geometry per step (i = 128c + m, j = 128k + p):
  - 16 main matmuls (c,k): lhsT = W chunk [128,128] fp16 (stationary),
    rhs = hT[:, 32k:32k+32] fp16 (moving), out = ps{A,B}[0:128, 32(c%2):+32].
  - 4 x-term matmuls (c): lhsT rows 0/1 = [w_ih; b_ih+b_hh] chunk, rows
    2..127 zero; rhs = xtwin ring slot [128,32] (row0 = x_t, row1 = ones).
  - 2 tanh activations [128,64]: psA -> hT[:,0:64], psB -> hT[:,64:128],
    interleaved with the matmul phases for cross-step pipelining.

PSUM discipline: start=True clears has_written for the WHOLE bank, and
PE-write + ScalarE-read of one bank is fatal.  So: 4 full-bank psum tiles
(A/B x ping-pong), all hot-loop matmuls use start=False (per-element
has_written gives overwrite-or-accumulate onto zeros), banks are zeroed by
VectorE after each tanh reads them, and one-time init matmuls (zero
weights) establish clean values at kernel start.

fp16 weights/activations (PSUM accumulation stays fp32): verified on CPU to
reproduce loss to 1e-5 and the correct-count exactly; measured on HW at
rel err 2e-6.

Final linear head (10 classes) on device; log-softmax / loss / argmax on
host (tiny [256,10] reduction).
"""

import sys

if "/opt/trn_rl_repo" not in sys.path:
    sys.path.insert(0, "/opt/trn_rl_repo")

import numpy as np

B, T, H, NCLS = 256, 784, 512, 10
NCORES = 8
BC = B // NCORES   # 32 samples per core
KC = H // 128      # 4 contraction chunks (k)
CC = H // 128      # 4 output chunks (c)
RING = 16          # xtwin ring depth (steps)

_BUILD_CACHE = {}


def _build(t_steps=T, split_waits=True):
    """Build the Bass module (single program, run SPMD on 8 cores)."""
    import concourse.bass as bass
    import concourse.mybir as mybir
    from concourse import tile

    f16 = mybir.dt.float16
    f32 = mybir.dt.float32
    Tanh = mybir.ActivationFunctionType.Tanh

    nc = bass.Bass(
        "TRN2",
        target_bir_lowering=False,
        debug=False,
        enable_asserts=False,
        num_devices=NCORES,
    )

    d_WTC = nc.dram_tensor("WTC", (128, CC * KC * 128), f16,
                           kind="ExternalInput").ap()
    d_wxC = nc.dram_tensor("wxC", (128, CC * 128), f16,
                           kind="ExternalInput").ap()
    d_xT = nc.dram_tensor("xT", (2, t_steps * BC), f16,
                          kind="ExternalInput").ap()
    d_lWTC = nc.dram_tensor("lWTC", (128, KC * NCLS), f16,
                            kind="ExternalInput").ap()
    d_out = nc.dram_tensor("logitsT", (NCLS, BC), f32, kind="ExternalOutput").ap()

    with tile.TileContext(nc) as tc:
        with (
            tc.tile_pool(name="const", bufs=1) as cpool,
            tc.tile_pool(name="ps", bufs=1, space="PSUM") as ppool,
        ):
            WTC = cpool.tile([128, CC * KC * 128], f16, tag="WTC")
            wxC = cpool.tile([128, CC * 128], f16, tag="wxC")
            xT = cpool.tile([2, t_steps * BC], f16, tag="xT")
            lWTC = cpool.tile([128, KC * NCLS], f16, tag="lWTC")
            xtwin = cpool.tile([128, RING * BC], f16, tag="xtwin")
            zerosW = cpool.tile([128, 128], f16, tag="zerosW")
            warm = cpool.tile([128, 1], f32, tag="warm")
            out_sb = cpool.tile([NCLS, BC], f32, tag="out")
            hh = [cpool.tile([128, CC * BC], f16, tag=f"h{p}", name=f"h{p}")
                  for p in range(2)]
            # 4 full-bank psum tiles: [A,B] x ping-pong parity. Full 512-f32
            # allocation guarantees bank exclusivity (PE-write vs ScE-read of
            # one bank is a fatal HW collision).
            ps = [[ppool.tile([128, 512], f32, tag=f"ps{ab}{p}",
                              name=f"ps{ab}{p}") for ab in range(2)]
                  for p in range(2)]

            half = CC * KC * 128 // 2
            nc.sync.dma_start(out=WTC[:, 0:half], in_=d_WTC[:, 0:half])
            nc.sync.dma_start(out=WTC[:, half:2 * half],
                              in_=d_WTC[:, half:2 * half])
            nc.sync.dma_start(out=wxC[:, :], in_=d_wxC)
            nc.sync.dma_start(out=xT[:, :], in_=d_xT)
            nc.sync.dma_start(out=lWTC[:, :], in_=d_lWTC)

            # xtwin ring: row0 = x_t, row1 = ones (both refreshed per window
            # from xT, whose row1 is host-packed ones), rows 2..127 = zeros
            # (x lhsT rows are zero there; still must be NaN-free).
            nc.vector.memset(xtwin[:, :], 0.0)
            nc.vector.memset(zerosW[:, :], 0.0)
            nc.vector.memset(warm[:, :], 0.0)

            def refresh(w):
                t0 = w * RING
                t1 = min(t_steps, t0 + RING)
                nc.vector.tensor_copy(
                    xtwin[0:2, 0:(t1 - t0) * BC],
                    xT[0:2, t0 * BC:t1 * BC],
                )

            refresh(0)  # also absorbs the xT DMA semaphore on the DVE

            # gate matmuls: absorb each PE-consumed DMA queue's semaphore
            # into the PE's observed clock (results discarded; unused psum
            # region of bank B0).
            for glhs, grhs in (
                (WTC[:, 0:128], WTC[:, 0:BC]),
                (WTC[:, half:half + 128], WTC[:, half:half + BC]),
                (wxC[:, 0:128], wxC[:, 0:BC]),
                (lWTC[:, 0:NCLS], lWTC[:, 0:BC]),
            ):
                nc.tensor.matmul(ps[0][1][0:glhs.shape[-1], 448:448 + BC],
                                 glhs, grhs, start=True, stop=True,
                                 tile_position=(0, 0))

            # one-time psum init: zero values into every region the loop
            # accumulates into, so the all-start=False hot loop is correct
            # regardless of has_written state (set -> accumulate onto 0,
            # clear -> overwrite).
            for p in range(2):
                for ab in range(2):
                    nc.tensor.matmul(
                        ps[p][ab][0:128, 0:2 * BC],
                        zerosW[:, :],
                        xtwin[:, 0:2 * BC],
                        start=True, stop=True, skip_group_check=True,
                        tile_position=(0, 0),
                    )

            def x_slot(t, p, c):
                nc.tensor.matmul(
                    ps[p][c // 2][0:128, 32 * (c % 2):32 * (c % 2) + 32],
                    wxC[:, c * 128:(c + 1) * 128],
                    xtwin[:, (t % RING) * BC:(t % RING + 1) * BC],
                    start=False, stop=False, skip_group_check=True,
                    tile_position=(0, 0),
                )

            def main_slot(t, p, c, k):
                nc.tensor.matmul(
                    ps[p][c // 2][0:128, 32 * (c % 2):32 * (c % 2) + 32],
                    WTC[:, (c * KC + k) * 128:(c * KC + k + 1) * 128],
                    hh[1 - p][:, k * BC:(k + 1) * BC],
                    start=False, stop=(k == KC - 1), skip_group_check=True,
                    tile_position=(0, 0),
                )

            for t in range(t_steps):
                p = t % 2
                if t % RING == 0 and t > 0:
                    refresh(t // RING)

                # x-slots first: no dependency on the previous tanh, so they
                # absorb the inter-step pipeline stall.
                for c in range(CC):
                    x_slot(t, p, c)
                if t > 0:
                    # Bank A (chunks 0,1) completes in its first 8 slots so
                    # tanhA -- the critical cross-step gate -- fires early.
                    # k<2 slots gate on tanhA(t-1), k>=2 on tanhB(t-1).
                    for c in (0, 1):
                        for k in (0, 1):
                            main_slot(t, p, c, k)
                    for c in (0, 1):
                        for k in (2, 3):
                            main_slot(t, p, c, k)
                    nc.scalar.activation(
                        hh[p][:, 0:2 * BC], ps[p][0][:, 0:2 * BC], Tanh)
                    nc.vector.memset(ps[p][0][:, 0:2 * BC], 0.0)
                    for c in (2, 3):
                        for k in (0, 1):
                            main_slot(t, p, c, k)
                    for c in (2, 3):
                        for k in (2, 3):
                            main_slot(t, p, c, k)
                    nc.scalar.activation(
                        hh[p][:, 2 * BC:4 * BC], ps[p][1][:, 0:2 * BC], Tanh)
                    nc.vector.memset(ps[p][1][:, 0:2 * BC], 0.0)
                    # dummy ScalarE op: keeps ACT "warm" through its idle
                    # window so the next tanhA runs at back-to-back cost
                    # (~203ns) instead of paying the ~110ns after-idle
                    # read-write bubble.
                    nc.scalar.copy(warm[:, :], warm[:, :])
                else:
                    nc.scalar.activation(
                        hh[p][:, 0:2 * BC], ps[p][0][:, 0:2 * BC], Tanh)
                    nc.vector.memset(ps[p][0][:, 0:2 * BC], 0.0)
                    nc.scalar.activation(
                        hh[p][:, 2 * BC:4 * BC], ps[p][1][:, 0:2 * BC], Tanh)
                    nc.vector.memset(ps[p][1][:, 0:2 * BC], 0.0)

            # final linear head: logitsT[c, b] = sum_j lin_W[c, j] h[b, j]
            # into an untouched region of bank A0 (bits clear since init ->
            # first matmul overwrites, rest accumulate).
            pl = (t_steps - 1) % 2
            for k in range(KC):
                nc.tensor.matmul(
                    ps[0][0][0:NCLS, 256:256 + BC],
                    lWTC[:, k * NCLS:(k + 1) * NCLS],
                    hh[pl][:, k * BC:(k + 1) * BC],
                    start=False, stop=(k == KC - 1), skip_group_check=True,
                    tile_position=(0, 0),
                )
            nc.vector.tensor_copy(out_sb[:, :], ps[0][0][0:NCLS, 256:256 + BC])
            nc.sync.dma_start(out=d_out, in_=out_sb[:, :])

    if split_waits:
        _split_multi_waits(nc, mybir)
    return nc


def _split_multi_waits(nc, mybir):
    """Walrus can pack only one sync wait into a HW instruction. Move any
    extra waits onto same-engine NoOps inserted right before (the engine's
    sequencer executes them in order, so semantics are unchanged)."""
    nid = 0
    for b in nc.m.functions[0].blocks:
        out = []
        changed = False
        for ins in b.instructions:
            si = getattr(ins, "sync_info", None)
            ws = list(getattr(si, "on_wait", []) or []) if si else []
            if len(ws) > 1:
                for w in ws[:-1]:
                    nid += 1
                    out.append(mybir.InstNoOp(
                        name=f"I-wsplit-{nid}",
                        engine=ins.engine,
                        sync_info=mybir.SyncInfo(on_wait=[w], on_update=[]),
                    ))
                ins.sync_info = mybir.SyncInfo(
                    on_wait=[ws[-1]], on_update=list(si.on_update or [])
                )
                changed = True
            out.append(ins)
        if changed:
            b.instructions = out


def _pack_inputs(inputs, order, W_ih, b_ih, W_hh, b_hh, lin_W, t_steps=T):
    """Host-side shard packing: returns in_maps list (one dict per core)."""
    x = np.asarray(inputs, np.float32)[:, np.asarray(order, np.int64)]
    x = np.ascontiguousarray(x[:, :t_steps]).astype(np.float16)
    W = np.asarray(W_hh, np.float32).astype(np.float16)
    wih = np.asarray(W_ih, np.float32)[:, 0].astype(np.float16)
    bias = (np.asarray(b_ih, np.float32)
            + np.asarray(b_hh, np.float32)).astype(np.float16)
    lW = np.asarray(lin_W, np.float32).astype(np.float16)

    # WTC[p, (c*KC+k)*128 + m] = W[i, j], i = 128c+m, j = 128k+p
    WTC = np.zeros((128, CC * KC * 128), np.float16)
    for c in range(CC):
        for k in range(KC):
            s = c * KC + k
            WTC[:, s * 128:(s + 1) * 128] = \
                W[128 * c:128 * (c + 1), 128 * k:128 * (k + 1)].T

    # wxC[0/1, c*128 + m] = w_ih/bias at i = 128c+m
    wxC = np.zeros((128, CC * 128), np.float16)
    for c in range(CC):
        wxC[0, c * 128:(c + 1) * 128] = wih[128 * c:128 * (c + 1)]
        wxC[1, c * 128:(c + 1) * 128] = bias[128 * c:128 * (c + 1)]

    # lWTC[p, k*NCLS + c] = lin_W[c, 128k + p]
    lWTC = np.zeros((128, KC * NCLS), np.float16)
    for k in range(KC):
        lWTC[:, k * NCLS:(k + 1) * NCLS] = lW[:, 128 * k:128 * (k + 1)].T

    in_maps = []
    for c in range(NCORES):
        xc = x[c * BC:(c + 1) * BC]                    # [BC, t]
        xTv = np.ones((2, t_steps * BC), np.float16)
        xTv[0] = xc.T.reshape(-1)
        in_maps.append(
            {"WTC": WTC, "wxC": wxC, "xT": xTv, "lWTC": lWTC}
        )
    return in_maps


def _run(inputs, y, order, W_ih, b_ih, W_hh, b_hh, lin_W, lin_b, trace=False):
    from concourse import bass_utils

    key = T
    if key not in _BUILD_CACHE:
        _BUILD_CACHE[key] = _build(T)
    nc = _BUILD_CACHE[key]

    in_maps = _pack_inputs(inputs, order, W_ih, b_ih, W_hh, b_hh, lin_W, T)
    res = bass_utils.run_bass_kernel_spmd(
        nc, in_maps, core_ids=list(range(NCORES)), trace=trace
    )

    logits = np.empty((B, NCLS), np.float32)
    for c in range(NCORES):
        logits[c * BC:(c + 1) * BC] = res.results[c]["logitsT"].T
    logits = logits + np.asarray(lin_b, np.float32)[None, :]

    yv = np.asarray(y).astype(np.int64)
    m = logits.max(axis=1, keepdims=True)
    logp = logits - (np.log(np.exp(logits - m).sum(axis=1, keepdims=True)) + m)
    loss = np.float32(-logp[np.arange(B), yv].mean())
    correct = np.int32((logits.argmax(axis=1) == yv).sum())
    return (loss, correct), res


def kernel(inputs, y, order, W_ih, b_ih, W_hh, b_hh, lin_W, lin_b):
    out, _ = _run(inputs, y, order, W_ih, b_ih, W_hh, b_hh, lin_W, lin_b)
    return out
